# revision 6
# baseline (speedup 1.0000x reference)
"""AdLIF neuron Bass kernel for 8 Trainium2 NeuronCores.

2-hop-dependency formulation in decay-rescaled space.  With
m_t = u_t - 2 (pre-reset shifted membrane) and K_t = alpha_mem^-t,
track n_t = m_t * K_t.  The recurrence becomes a running sum:
    F_t = n_{t-1} + K_t * x~_t          x~ = x + 2*(alpha_mem - 1)
    n_t = F_t - K_{t-1} * s_{t-1}       (reset)
    s_t = (0.1 * K_t * a_{t-1} <= n_t)  (<=> u_t >= 0.1*a + 2)
    a_t = alpha_adp * a_{t-1} + s_t
All K powers enter as per-instruction scalar immediates, so every
dependency cycle through the spike decision is 2 instructions long
(F depends only on n, not on s).  Numerically validated vs the jax
reference (0 mismatches in fp32 emulation on the actual input).
The x~ pre-shift runs on the Scalar (Activation) engine per chunk,
hidden behind the DVE recurrence.

Sharding: D (1024) split across 8 cores -> 128 d's per core.
Per core the 32*128 = 4096 (b,d) elements are laid out as
[eh=128 partitions, el=32 free] and time runs in the free dim of a
[128, T*32] SBUF buffer, so each timestep is one [128, 32] slice.
Host pre-reshapes x to [core, eh, t, el] so the per-core DMA is one
fully contiguous 64KB-per-partition stream.
"""

import os
import numpy as np
from contextlib import ExitStack

import concourse.bass as bass
import concourse.tile as tile
from concourse import bacc, mybir
from concourse.bass_utils import run_bass_kernel_spmd

B, T, D = 32, 512, 1024
NCORES = 8
DLOC = D // NCORES          # 128 d's per core
EH, EL = 128, 32            # 4096 elements per core = EH partitions x EL free
# Uneven t-chunks: small first chunk so compute starts after a ~1MB DMA,
# small last chunk so the final output DMA tail is short.
CHUNKS = [16, 240, 240, 16]
NCHUNK = len(CHUNKS)
CSTART = [sum(CHUNKS[:i]) for i in range(NCHUNK)]

PAD = 16                    # trailing dummy cols on tight-pair producers

ALPHA_MEM = float(np.exp(-1.0 / 20.0))
ALPHA_ADP = float(np.exp(-1.0 / 200.0))
# x~ bias: fold the compare constant 2 into the membrane state.
XBIAS = float(np.float32(2.0 * np.float32(ALPHA_MEM) - 2.0))
# K_t = alpha_mem^-t at fp32 (matches the numpy validation exactly).
KPOW = (np.float64(np.float32(ALPHA_MEM)) ** (-np.arange(T))).astype(np.float32)
# n_{-1} = g_init * K_{-1} = (-2) * alpha_mem
NINIT = float(np.float32(-2.0) * np.float32(ALPHA_MEM))

LAST_RESULT = None  # BassKernelResults of the most recent run (for test.py)

F32 = mybir.dt.float32
OP = mybir.AluOpType


def _build():
    nc = bacc.Bacc("TRN2", target_bir_lowering=False, debug=False)
    x_ext = nc.declare_dram_parameter("x", [EH, T * EL], F32, isOutput=False)
    s_ext = nc.declare_dram_parameter("out", [EH, T * EL], F32, isOutput=True)

    with tile.TileContext(nc) as tc, ExitStack() as ctx:
        data = ctx.enter_context(tc.tile_pool(name="data", bufs=1))
        xin = [data.tile([EH, CHUNKS[k] * EL + PAD], F32, name=f"xin{k}", tag=f"x{k}")
               for k in range(NCHUNK)]
        sout = [data.tile([EH, CHUNKS[k] * EL + PAD], F32, name=f"sout{k}", tag=f"s{k}")
                for k in range(NCHUNK)]

        st = ctx.enter_context(tc.tile_pool(name="state", bufs=1))
        n = st.tile([EH, EL + PAD], F32, tag="n")
        a = st.tile([EH, EL + PAD], F32, tag="a")
        fb = st.tile([EH, EL + PAD], F32, tag="fb")
        szero = st.tile([EH, EL + PAD], F32, tag="szero")
        bias = st.tile([EH, 1], F32, tag="bias")
        nc.vector.memset(bias[:], XBIAS)

        for k in range(NCHUNK):
            nc.gpsimd.dma_start(
                xin[k][:, 0:CHUNKS[k] * EL],
                x_ext[:, CSTART[k] * EL:(CSTART[k] + CHUNKS[k]) * EL])
            nc.vector.memset(xin[k][:, CHUNKS[k] * EL:], 0.0)
            # x~ = x + c0 on the Scalar engine (own SBUF port, otherwise
            # idle); one instruction per chunk, pipelined ahead of the DVE.
            nc.scalar.activation(xin[k][:, 0:CHUNKS[k] * EL],
                                 xin[k][:, 0:CHUNKS[k] * EL],
                                 mybir.ActivationFunctionType.Identity,
                                 bias=bias[:], scale=1.0)

        nc.vector.memset(n[:], NINIT)
        nc.vector.memset(a[:], 0.0)
        nc.vector.memset(szero[:], 0.0)

        # Per-step group [F(t), A(t-1), R(t), CMP(t)].  The only tight
        # (adjacent-instruction) same-engine dependency is R->CMP, whose
        # producer streams PAD extra dummy columns so its real writebacks
        # retire before CMP's reads reach them -- replacing semaphore
        # waits.  All other deps are >=2 instructions back, where the DVE
        # pipeline overlap can no longer race (empirically validated), so
        # their semaphore waits are stripped below.
        def slot(t):
            k = next(i for i in range(NCHUNK)
                     if CSTART[i] <= t < CSTART[i] + CHUNKS[i])
            return k, t - CSTART[k]

        for t in range(T):
            k, j = slot(t)
            xt_pad = xin[k][:, j * EL:(j + 1) * EL + PAD]
            st_ = sout[k][:, j * EL:(j + 1) * EL]
            if t == 0:
                sprev_pad = szero[:]
            else:
                kp, jp = slot(t - 1)
                sprev_pad = sout[kp][:, jp * EL:jp * EL + EL + PAD]
            kt = float(KPOW[t])
            kprev = float(ALPHA_MEM) if t == 0 else float(KPOW[t - 1])
            ck = float(np.float32(np.float32(0.1) * KPOW[t]))

            # F = K_t * x~_t + n_{t-1}   (off the spike cycle; FD=EL only --
            # R reads fb's stale pad cols, which is harmless garbage)
            nc.vector.scalar_tensor_tensor(fb[:, 0:EL], xin[k][:, j * EL:(j + 1) * EL],
                                           kt, n[:, 0:EL],
                                           op0=OP.mult, op1=OP.add)
            # a-update for the previous step (s_{t-1} is 2 insts back)
            if t > 0:
                nc.vector.scalar_tensor_tensor(a[:, 0:EL], a[:, 0:EL],
                                               ALPHA_ADP,
                                               sout[kp][:, jp * EL:(jp + 1) * EL],
                                               op0=OP.mult, op1=OP.add)
            # n = F - K_{t-1} * s_{t-1}  (reset; streams PAD cols so CMP
            # can follow immediately without a wait)
            nc.vector.scalar_tensor_tensor(n[:], sprev_pad, -kprev, fb[:],
                                           op0=OP.mult, op1=OP.add)
            # s = (0.1*K_t*a <= n) -> output buffer
            nc.vector.scalar_tensor_tensor(st_, a[:, 0:EL], ck, n[:, 0:EL],
                                           op0=OP.mult, op1=OP.is_le)

            if j == CHUNKS[k] - 1:
                nc.gpsimd.dma_start(
                    s_ext[:, CSTART[k] * EL:(CSTART[k] + CHUNKS[k]) * EL],
                    sout[k][:, 0:CHUNKS[k] * EL])

    _strip_dve_sem_overhead(nc)
    nc.finalize()
    return nc


def _strip_dve_sem_overhead(nc):
    # The DVE overlaps at most the next instruction with the current one,
    # so a RAW hazard only exists between ADJACENT DVE instructions, and
    # the PAD trailing columns on the producer of the single tight pair
    # (R->CMP) delay the consumer's reads past the producer's writebacks.
    # That makes every Tile-emitted DVE-on-DVE semaphore wait (~180ns
    # event-propagation latency each) redundant -- strip them all.
    # Cross-engine waits (DMA/ACT<->DVE) and all semaphore updates are kept.
    for f in nc.m.functions:
        for bb in f.blocks:
            for inst in bb.instructions:
                if inst.engine != mybir.EngineType.DVE:
                    continue
                si = inst.sync_info
                if si is not None and si.on_wait:
                    kept = [w for w in si.on_wait
                            if not str(w.ant_name).startswith("DVE")]
                    if len(kept) != len(si.on_wait):
                        si.on_wait = kept

    # Of the ~2050 DVE semaphore updates only a handful of cumulative
    # threshold values are ever awaited (output DMAs, kernel-tail drain,
    # barrier event-semaphores).  Drop the updates nobody waits for and
    # remap the awaited thresholds to the compressed count, removing the
    # per-instruction semaphore-update overhead from the hot loop.
    insts = [i for f in nc.m.functions for bb in f.blocks for i in bb.instructions]

    def dve_sem_names(entries):
        return {str(e.ant_name) for e in entries if str(e.ant_name).startswith("DVE")}

    sems = set()
    for i in insts:
        if i.sync_info:
            sems |= dve_sem_names(i.sync_info.on_update or [])
    for sem in sems:
        awaited = set()
        for i in insts:
            si = i.sync_info
            if si is None:
                continue
            for wt in (si.on_wait or []):
                if str(wt.ant_name) == sem:
                    awaited.add(wt.wait_value)
        ordinal = 0
        remap = {}
        kept_count = 0
        for i in insts:
            si = i.sync_info
            if si is None:
                continue
            ups = [u for u in (si.on_update or []) if str(u.ant_name) == sem]
            if not ups:
                continue
            ordinal += 1
            if ordinal in awaited:
                kept_count += 1
                remap[ordinal] = kept_count
            else:
                si.on_update = [u for u in si.on_update
                                if str(u.ant_name) != sem]
        for i in insts:
            si = i.sync_info
            if si is None:
                continue
            for wt in (si.on_wait or []):
                if str(wt.ant_name) == sem:
                    wt.wait_value = remap[wt.wait_value]


def kernel(x: np.ndarray) -> np.ndarray:
    global LAST_RESULT
    x = np.ascontiguousarray(x, dtype=np.float32)
    assert x.shape == (B, T, D)

    # shard: core c owns d in [c*DLOC, (c+1)*DLOC); element (b, dh, dl):
    # eh = b*4 + dh, el = dl  with d = c*128 + dh*32 + dl
    xs = (x.reshape(B, T, NCORES, EH // B, EL)
           .transpose(2, 0, 3, 1, 4)
           .reshape(NCORES, EH, T * EL))

    nc = _build()
    in_maps = [{"x": np.ascontiguousarray(xs[c])} for c in range(NCORES)]
    LAST_RESULT = run_bass_kernel_spmd(
        nc, in_maps, list(range(NCORES)),
        trace=bool(os.environ.get("ADLIF_TRACE")),
    )
    outs = np.stack([LAST_RESULT.results[c]["out"] for c in range(NCORES)])

    s = (outs.reshape(NCORES, B, EH // B, T, EL)
             .transpose(1, 3, 0, 2, 4)
             .reshape(B, T, D))
    return np.ascontiguousarray(s, dtype=np.float32)


# revision 7
# speedup vs baseline: 1.0460x; 1.0460x over previous
"""AdLIF neuron Bass kernel for 8 Trainium2 NeuronCores.

2-hop-dependency formulation in decay-rescaled space.  With
m_t = u_t - 2 (pre-reset shifted membrane) and K_t = alpha_mem^-t,
track n_t = m_t * K_t.  The recurrence becomes a running sum:
    F_t = n_{t-1} + K_t * x~_t          x~ = x + 2*(alpha_mem - 1)
    n_t = F_t - K_{t-1} * s_{t-1}       (reset)
    s_t = (0.1 * K_t * a_{t-1} <= n_t)  (<=> u_t >= 0.1*a + 2)
    a_t = alpha_adp * a_{t-1} + s_t
All K powers enter as per-instruction scalar immediates, so every
dependency cycle through the spike decision is 2 instructions long
(F depends only on n, not on s).  Numerically validated vs the jax
reference (0 mismatches in fp32 emulation on the actual input).
The x~ pre-shift runs on the Scalar (Activation) engine per chunk,
hidden behind the DVE recurrence.

Sharding: D (1024) split across 8 cores -> 128 d's per core.
Per core the 32*128 = 4096 (b,d) elements are laid out as
[eh=128 partitions, el=32 free] and time runs in the free dim of a
[128, T*32] SBUF buffer, so each timestep is one [128, 32] slice.
Host pre-reshapes x to [core, eh, t, el] so the per-core DMA is one
fully contiguous 64KB-per-partition stream.
"""

import os
import numpy as np
from contextlib import ExitStack

import concourse.bass as bass
import concourse.tile as tile
from concourse import bacc, mybir
from concourse.bass_utils import run_bass_kernel_spmd

B, T, D = 32, 512, 1024
NCORES = 8
DLOC = D // NCORES          # 128 d's per core
EH, EL = 128, 32            # 4096 elements per core = EH partitions x EL free
# Uneven t-chunks: small first chunk so compute starts after a ~1MB DMA,
# small last chunk so the final output DMA tail is short.
CHUNKS = [32, 232, 232, 16]
NCHUNK = len(CHUNKS)
CSTART = [sum(CHUNKS[:i]) for i in range(NCHUNK)]

PAD = 16                    # trailing dummy cols on tight-pair producers

ALPHA_MEM = float(np.exp(-1.0 / 20.0))
ALPHA_ADP = float(np.exp(-1.0 / 200.0))
# x~ bias: fold the compare constant 2 into the membrane state.
XBIAS = float(np.float32(2.0 * np.float32(ALPHA_MEM) - 2.0))
# K_t = alpha_mem^-t at fp32 (matches the numpy validation exactly).
KPOW = (np.float64(np.float32(ALPHA_MEM)) ** (-np.arange(T))).astype(np.float32)
# n_{-1} = g_init * K_{-1} = (-2) * alpha_mem
NINIT = float(np.float32(-2.0) * np.float32(ALPHA_MEM))

LAST_RESULT = None  # BassKernelResults of the most recent run (for test.py)

F32 = mybir.dt.float32
OP = mybir.AluOpType


def _build():
    nc = bacc.Bacc("TRN2", target_bir_lowering=False, debug=False)
    x_ext = nc.declare_dram_parameter("x", [EH, T * EL], F32, isOutput=False)
    s_ext = nc.declare_dram_parameter("out", [EH, T * EL], F32, isOutput=True)

    with tile.TileContext(nc) as tc, ExitStack() as ctx:
        data = ctx.enter_context(tc.tile_pool(name="data", bufs=1))
        xin = [data.tile([EH, CHUNKS[k] * EL + PAD], F32, name=f"xin{k}", tag=f"x{k}")
               for k in range(NCHUNK)]
        sout = [data.tile([EH, CHUNKS[k] * EL + PAD], F32, name=f"sout{k}", tag=f"s{k}")
                for k in range(NCHUNK)]

        st = ctx.enter_context(tc.tile_pool(name="state", bufs=1))
        n = st.tile([EH, EL + PAD], F32, tag="n")
        a = st.tile([EH, EL + PAD], F32, tag="a")
        fb = st.tile([EH, EL + PAD], F32, tag="fb")
        szero = st.tile([EH, EL + PAD], F32, tag="szero")
        bias = st.tile([EH, 1], F32, tag="bias")
        nc.vector.memset(bias[:], XBIAS)

        for k in range(NCHUNK):
            nc.gpsimd.dma_start(
                xin[k][:, 0:CHUNKS[k] * EL],
                x_ext[:, CSTART[k] * EL:(CSTART[k] + CHUNKS[k]) * EL])
            nc.vector.memset(xin[k][:, CHUNKS[k] * EL:], 0.0)
            # x~ = x + c0 on the Scalar engine (own SBUF port, otherwise
            # idle); one instruction per chunk, pipelined ahead of the DVE.
            nc.scalar.activation(xin[k][:, 0:CHUNKS[k] * EL],
                                 xin[k][:, 0:CHUNKS[k] * EL],
                                 mybir.ActivationFunctionType.Identity,
                                 bias=bias[:], scale=1.0)

        nc.vector.memset(n[:], NINIT)
        nc.vector.memset(a[:], 0.0)
        nc.vector.memset(szero[:], 0.0)

        # Per-step group [F(t), A(t-1), R(t), CMP(t)].  The only tight
        # (adjacent-instruction) same-engine dependency is R->CMP, whose
        # producer streams PAD extra dummy columns so its real writebacks
        # retire before CMP's reads reach them -- replacing semaphore
        # waits.  All other deps are >=2 instructions back, where the DVE
        # pipeline overlap can no longer race (empirically validated), so
        # their semaphore waits are stripped below.
        def slot(t):
            k = next(i for i in range(NCHUNK)
                     if CSTART[i] <= t < CSTART[i] + CHUNKS[i])
            return k, t - CSTART[k]

        for t in range(T):
            k, j = slot(t)
            xt_pad = xin[k][:, j * EL:(j + 1) * EL + PAD]
            st_ = sout[k][:, j * EL:(j + 1) * EL]
            if t == 0:
                sprev_pad = szero[:]
            else:
                kp, jp = slot(t - 1)
                sprev_pad = sout[kp][:, jp * EL:jp * EL + EL + PAD]
            kt = float(KPOW[t])
            kprev = float(ALPHA_MEM) if t == 0 else float(KPOW[t - 1])
            ck = float(np.float32(np.float32(0.1) * KPOW[t]))

            # F = K_t * x~_t + n_{t-1}   (off the spike cycle; FD=EL only --
            # R reads fb's stale pad cols, which is harmless garbage)
            nc.vector.scalar_tensor_tensor(fb[:, 0:EL], xin[k][:, j * EL:(j + 1) * EL],
                                           kt, n[:, 0:EL],
                                           op0=OP.mult, op1=OP.add)
            # a-update for the previous step (s_{t-1} is 2 insts back)
            if t > 0:
                nc.vector.scalar_tensor_tensor(a[:, 0:EL], a[:, 0:EL],
                                               ALPHA_ADP,
                                               sout[kp][:, jp * EL:(jp + 1) * EL],
                                               op0=OP.mult, op1=OP.add)
            # n = F - K_{t-1} * s_{t-1}  (reset; streams PAD cols so CMP
            # can follow immediately without a wait)
            nc.vector.scalar_tensor_tensor(n[:], sprev_pad, -kprev, fb[:],
                                           op0=OP.mult, op1=OP.add)
            # s = (0.1*K_t*a <= n) -> output buffer
            nc.vector.scalar_tensor_tensor(st_, a[:, 0:EL], ck, n[:, 0:EL],
                                           op0=OP.mult, op1=OP.is_le)

            if j == CHUNKS[k] - 1:
                nc.gpsimd.dma_start(
                    s_ext[:, CSTART[k] * EL:(CSTART[k] + CHUNKS[k]) * EL],
                    sout[k][:, 0:CHUNKS[k] * EL])

    _strip_dve_sem_overhead(nc)
    nc.finalize()
    return nc


def _strip_dve_sem_overhead(nc):
    # The DVE overlaps at most the next instruction with the current one,
    # so a RAW hazard only exists between ADJACENT DVE instructions, and
    # the PAD trailing columns on the producer of the single tight pair
    # (R->CMP) delay the consumer's reads past the producer's writebacks.
    # That makes every Tile-emitted DVE-on-DVE semaphore wait (~180ns
    # event-propagation latency each) redundant -- strip them all.
    # Cross-engine waits (DMA/ACT<->DVE) and all semaphore updates are kept.
    for f in nc.m.functions:
        for bb in f.blocks:
            for inst in bb.instructions:
                if inst.engine != mybir.EngineType.DVE:
                    continue
                si = inst.sync_info
                if si is not None and si.on_wait:
                    kept = [w for w in si.on_wait
                            if not str(w.ant_name).startswith("DVE")]
                    if len(kept) != len(si.on_wait):
                        si.on_wait = kept

    # Of the ~2050 DVE semaphore updates only a handful of cumulative
    # threshold values are ever awaited (output DMAs, kernel-tail drain,
    # barrier event-semaphores).  Drop the updates nobody waits for and
    # remap the awaited thresholds to the compressed count, removing the
    # per-instruction semaphore-update overhead from the hot loop.
    insts = [i for f in nc.m.functions for bb in f.blocks for i in bb.instructions]

    def dve_sem_names(entries):
        return {str(e.ant_name) for e in entries if str(e.ant_name).startswith("DVE")}

    sems = set()
    for i in insts:
        if i.sync_info:
            sems |= dve_sem_names(i.sync_info.on_update or [])
    for sem in sems:
        awaited = set()
        for i in insts:
            si = i.sync_info
            if si is None:
                continue
            for wt in (si.on_wait or []):
                if str(wt.ant_name) == sem:
                    awaited.add(wt.wait_value)
        ordinal = 0
        remap = {}
        kept_count = 0
        for i in insts:
            si = i.sync_info
            if si is None:
                continue
            ups = [u for u in (si.on_update or []) if str(u.ant_name) == sem]
            if not ups:
                continue
            ordinal += 1
            if ordinal in awaited:
                kept_count += 1
                remap[ordinal] = kept_count
            else:
                si.on_update = [u for u in si.on_update
                                if str(u.ant_name) != sem]
        for i in insts:
            si = i.sync_info
            if si is None:
                continue
            for wt in (si.on_wait or []):
                if str(wt.ant_name) == sem:
                    wt.wait_value = remap[wt.wait_value]


def kernel(x: np.ndarray) -> np.ndarray:
    global LAST_RESULT
    x = np.ascontiguousarray(x, dtype=np.float32)
    assert x.shape == (B, T, D)

    # shard: core c owns d in [c*DLOC, (c+1)*DLOC); element (b, dh, dl):
    # eh = b*4 + dh, el = dl  with d = c*128 + dh*32 + dl
    xs = (x.reshape(B, T, NCORES, EH // B, EL)
           .transpose(2, 0, 3, 1, 4)
           .reshape(NCORES, EH, T * EL))

    nc = _build()
    in_maps = [{"x": np.ascontiguousarray(xs[c])} for c in range(NCORES)]
    LAST_RESULT = run_bass_kernel_spmd(
        nc, in_maps, list(range(NCORES)),
        trace=bool(os.environ.get("ADLIF_TRACE")),
    )
    outs = np.stack([LAST_RESULT.results[c]["out"] for c in range(NCORES)])

    s = (outs.reshape(NCORES, B, EH // B, T, EL)
             .transpose(1, 3, 0, 2, 4)
             .reshape(B, T, D))
    return np.ascontiguousarray(s, dtype=np.float32)


# revision 10
# speedup vs baseline: 1.0496x; 1.0035x over previous
"""AdLIF neuron Bass kernel for 8 Trainium2 NeuronCores.

2-hop-dependency formulation in decay-rescaled space.  With
m_t = u_t - 2 (pre-reset shifted membrane) and K_t = alpha_mem^-t,
track n_t = m_t * K_t.  The recurrence becomes a running sum:
    F_t = n_{t-1} + K_t * x~_t          x~ = x + 2*(alpha_mem - 1)
    n_t = F_t - K_{t-1} * s_{t-1}       (reset)
    s_t = (0.1 * K_t * a_{t-1} <= n_t)  (<=> u_t >= 0.1*a + 2)
    a_t = alpha_adp * a_{t-1} + s_t
All K powers enter as per-instruction scalar immediates, so every
dependency cycle through the spike decision is 2 instructions long
(F depends only on n, not on s).  Numerically validated vs the jax
reference (0 mismatches in fp32 emulation on the actual input).
The x~ pre-shift runs on the Scalar (Activation) engine per chunk,
hidden behind the DVE recurrence.

Sharding: D (1024) split across 8 cores -> 128 d's per core.
Per core the 32*128 = 4096 (b,d) elements are laid out as
[eh=128 partitions, el=32 free] and time runs in the free dim of a
[128, T*32] SBUF buffer, so each timestep is one [128, 32] slice.
Host pre-reshapes x to [core, eh, t, el] so the per-core DMA is one
fully contiguous 64KB-per-partition stream.
"""

import os
import numpy as np
from contextlib import ExitStack

import concourse.bass as bass
import concourse.tile as tile
from concourse import bacc, mybir
from concourse.bass_utils import run_bass_kernel_spmd

B, T, D = 32, 512, 1024
NCORES = 8
DLOC = D // NCORES          # 128 d's per core
EH, EL = 128, 32            # 4096 elements per core = EH partitions x EL free
# Uneven t-chunks: small first chunk so compute starts after a ~1MB DMA,
# small last chunk so the final output DMA tail is short.
CHUNKS = [32, 232, 232, 16]
NCHUNK = len(CHUNKS)
CSTART = [sum(CHUNKS[:i]) for i in range(NCHUNK)]

PAD = 8                     # trailing dummy cols on tight-pair producers

ALPHA_MEM = float(np.exp(-1.0 / 20.0))
ALPHA_ADP = float(np.exp(-1.0 / 200.0))
# x~ bias: fold the compare constant 2 into the membrane state.
XBIAS = float(np.float32(2.0 * np.float32(ALPHA_MEM) - 2.0))
# K_t = alpha_mem^-t at fp32 (matches the numpy validation exactly).
KPOW = (np.float64(np.float32(ALPHA_MEM)) ** (-np.arange(T))).astype(np.float32)
# n_{-1} = g_init * K_{-1} = (-2) * alpha_mem
NINIT = float(np.float32(-2.0) * np.float32(ALPHA_MEM))

LAST_RESULT = None  # BassKernelResults of the most recent run (for test.py)

F32 = mybir.dt.float32
OP = mybir.AluOpType


def _build():
    nc = bacc.Bacc("TRN2", target_bir_lowering=False, debug=False)
    x_ext = nc.declare_dram_parameter("x", [EH, T * EL], F32, isOutput=False)
    s_ext = nc.declare_dram_parameter("out", [EH, T * EL], F32, isOutput=True)

    with tile.TileContext(nc) as tc, ExitStack() as ctx:
        data = ctx.enter_context(tc.tile_pool(name="data", bufs=1))
        xin = [data.tile([EH, CHUNKS[k] * EL + PAD], F32, name=f"xin{k}", tag=f"x{k}")
               for k in range(NCHUNK)]
        sout = [data.tile([EH, CHUNKS[k] * EL + PAD], F32, name=f"sout{k}", tag=f"s{k}")
                for k in range(NCHUNK)]

        st = ctx.enter_context(tc.tile_pool(name="state", bufs=1))
        n = st.tile([EH, EL + PAD], F32, tag="n")
        a = st.tile([EH, EL + PAD], F32, tag="a")
        fb = st.tile([EH, EL + PAD], F32, tag="fb")
        szero = st.tile([EH, EL + PAD], F32, tag="szero")
        bias = st.tile([EH, 1], F32, tag="bias")
        nc.vector.memset(bias[:], XBIAS)

        for k in range(NCHUNK):
            nc.sync.dma_start(
                xin[k][:, 0:CHUNKS[k] * EL],
                x_ext[:, CSTART[k] * EL:(CSTART[k] + CHUNKS[k]) * EL])
            nc.vector.memset(xin[k][:, CHUNKS[k] * EL:], 0.0)
            # x~ = x + c0 on the Scalar engine (own SBUF port, otherwise
            # idle); one instruction per chunk, pipelined ahead of the DVE.
            nc.scalar.activation(xin[k][:, 0:CHUNKS[k] * EL],
                                 xin[k][:, 0:CHUNKS[k] * EL],
                                 mybir.ActivationFunctionType.Identity,
                                 bias=bias[:], scale=1.0)

        nc.vector.memset(n[:], NINIT)
        nc.vector.memset(a[:], 0.0)
        nc.vector.memset(szero[:], 0.0)

        # Per-step group [F(t), A(t-1), R(t), CMP(t)].  The only tight
        # (adjacent-instruction) same-engine dependency is R->CMP, whose
        # producer streams PAD extra dummy columns so its real writebacks
        # retire before CMP's reads reach them -- replacing semaphore
        # waits.  All other deps are >=2 instructions back, where the DVE
        # pipeline overlap can no longer race (empirically validated), so
        # their semaphore waits are stripped below.
        def slot(t):
            k = next(i for i in range(NCHUNK)
                     if CSTART[i] <= t < CSTART[i] + CHUNKS[i])
            return k, t - CSTART[k]

        for t in range(T):
            k, j = slot(t)
            xt_pad = xin[k][:, j * EL:(j + 1) * EL + PAD]
            st_ = sout[k][:, j * EL:(j + 1) * EL]
            if t == 0:
                sprev_pad = szero[:]
            else:
                kp, jp = slot(t - 1)
                sprev_pad = sout[kp][:, jp * EL:jp * EL + EL + PAD]
            kt = float(KPOW[t])
            kprev = float(ALPHA_MEM) if t == 0 else float(KPOW[t - 1])
            ck = float(np.float32(np.float32(0.1) * KPOW[t]))

            # F = K_t * x~_t + n_{t-1}   (off the spike cycle; FD=EL only --
            # R reads fb's stale pad cols, which is harmless garbage)
            nc.vector.scalar_tensor_tensor(fb[:, 0:EL], xin[k][:, j * EL:(j + 1) * EL],
                                           kt, n[:, 0:EL],
                                           op0=OP.mult, op1=OP.add)
            # a-update for the previous step (s_{t-1} is 2 insts back)
            if t > 0:
                nc.vector.scalar_tensor_tensor(a[:, 0:EL], a[:, 0:EL],
                                               ALPHA_ADP,
                                               sout[kp][:, jp * EL:(jp + 1) * EL],
                                               op0=OP.mult, op1=OP.add)
            # n = F - K_{t-1} * s_{t-1}  (reset; streams PAD cols so CMP
            # can follow immediately without a wait)
            nc.vector.scalar_tensor_tensor(n[:], sprev_pad, -kprev, fb[:],
                                           op0=OP.mult, op1=OP.add)
            # s = (0.1*K_t*a <= n) -> output buffer
            nc.vector.scalar_tensor_tensor(st_, a[:, 0:EL], ck, n[:, 0:EL],
                                           op0=OP.mult, op1=OP.is_le)

            if j == CHUNKS[k] - 1:
                nc.sync.dma_start(
                    s_ext[:, CSTART[k] * EL:(CSTART[k] + CHUNKS[k]) * EL],
                    sout[k][:, 0:CHUNKS[k] * EL])

    _strip_dve_sem_overhead(nc)
    nc.finalize()
    return nc


def _strip_dve_sem_overhead(nc):
    # The DVE overlaps at most the next instruction with the current one,
    # so a RAW hazard only exists between ADJACENT DVE instructions, and
    # the PAD trailing columns on the producer of the single tight pair
    # (R->CMP) delay the consumer's reads past the producer's writebacks.
    # That makes every Tile-emitted DVE-on-DVE semaphore wait (~180ns
    # event-propagation latency each) redundant -- strip them all.
    # Cross-engine waits (DMA/ACT<->DVE) and all semaphore updates are kept.
    for f in nc.m.functions:
        for bb in f.blocks:
            for inst in bb.instructions:
                if inst.engine != mybir.EngineType.DVE:
                    continue
                si = inst.sync_info
                if si is not None and si.on_wait:
                    kept = [w for w in si.on_wait
                            if not str(w.ant_name).startswith("DVE")]
                    if len(kept) != len(si.on_wait):
                        si.on_wait = kept

    # Of the ~2050 DVE semaphore updates only a handful of cumulative
    # threshold values are ever awaited (output DMAs, kernel-tail drain,
    # barrier event-semaphores).  Drop the updates nobody waits for and
    # remap the awaited thresholds to the compressed count, removing the
    # per-instruction semaphore-update overhead from the hot loop.
    insts = [i for f in nc.m.functions for bb in f.blocks for i in bb.instructions]

    def dve_sem_names(entries):
        return {str(e.ant_name) for e in entries if str(e.ant_name).startswith("DVE")}

    sems = set()
    for i in insts:
        if i.sync_info:
            sems |= dve_sem_names(i.sync_info.on_update or [])
    for sem in sems:
        awaited = set()
        for i in insts:
            si = i.sync_info
            if si is None:
                continue
            for wt in (si.on_wait or []):
                if str(wt.ant_name) == sem:
                    awaited.add(wt.wait_value)
        ordinal = 0
        remap = {}
        kept_count = 0
        for i in insts:
            si = i.sync_info
            if si is None:
                continue
            ups = [u for u in (si.on_update or []) if str(u.ant_name) == sem]
            if not ups:
                continue
            ordinal += 1
            if ordinal in awaited:
                kept_count += 1
                remap[ordinal] = kept_count
            else:
                si.on_update = [u for u in si.on_update
                                if str(u.ant_name) != sem]
        for i in insts:
            si = i.sync_info
            if si is None:
                continue
            for wt in (si.on_wait or []):
                if str(wt.ant_name) == sem:
                    wt.wait_value = remap[wt.wait_value]


def kernel(x: np.ndarray) -> np.ndarray:
    global LAST_RESULT
    x = np.ascontiguousarray(x, dtype=np.float32)
    assert x.shape == (B, T, D)

    # shard: core c owns d in [c*DLOC, (c+1)*DLOC); element (b, dh, dl):
    # eh = b*4 + dh, el = dl  with d = c*128 + dh*32 + dl
    xs = (x.reshape(B, T, NCORES, EH // B, EL)
           .transpose(2, 0, 3, 1, 4)
           .reshape(NCORES, EH, T * EL))

    nc = _build()
    in_maps = [{"x": np.ascontiguousarray(xs[c])} for c in range(NCORES)]
    LAST_RESULT = run_bass_kernel_spmd(
        nc, in_maps, list(range(NCORES)),
        trace=bool(os.environ.get("ADLIF_TRACE")),
    )
    outs = np.stack([LAST_RESULT.results[c]["out"] for c in range(NCORES)])

    s = (outs.reshape(NCORES, B, EH // B, T, EL)
             .transpose(1, 3, 0, 2, 4)
             .reshape(B, T, D))
    return np.ascontiguousarray(s, dtype=np.float32)


# revision 12
# speedup vs baseline: 1.0500x; 1.0004x over previous
"""AdLIF neuron Bass kernel for 8 Trainium2 NeuronCores.

2-hop-dependency formulation in decay-rescaled space.  With
m_t = u_t - 2 (pre-reset shifted membrane) and K_t = alpha_mem^-t,
track n_t = m_t * K_t.  The recurrence becomes a running sum:
    F_t = n_{t-1} + K_t * x~_t          x~ = x + 2*(alpha_mem - 1)
    n_t = F_t - K_{t-1} * s_{t-1}       (reset)
    s_t = (0.1 * K_t * a_{t-1} <= n_t)  (<=> u_t >= 0.1*a + 2)
    a_t = alpha_adp * a_{t-1} + s_t
All K powers enter as per-instruction scalar immediates, so every
dependency cycle through the spike decision is 2 instructions long
(F depends only on n, not on s).  Numerically validated vs the jax
reference (0 mismatches in fp32 emulation on the actual input).
The x~ pre-shift runs on the Scalar (Activation) engine per chunk,
hidden behind the DVE recurrence.

Sharding: D (1024) split across 8 cores -> 128 d's per core.
Per core the 32*128 = 4096 (b,d) elements are laid out as
[eh=128 partitions, el=32 free] and time runs in the free dim of a
[128, T*32] SBUF buffer, so each timestep is one [128, 32] slice.
Host pre-reshapes x to [core, eh, t, el] so the per-core DMA is one
fully contiguous 64KB-per-partition stream.
"""

import os
import numpy as np
from contextlib import ExitStack

import concourse.bass as bass
import concourse.tile as tile
from concourse import bacc, mybir
from concourse.bass_utils import run_bass_kernel_spmd

B, T, D = 32, 512, 1024
NCORES = 8
DLOC = D // NCORES          # 128 d's per core
EH, EL = 128, 32            # 4096 elements per core = EH partitions x EL free
# Uneven t-chunks: small first chunk so compute starts after a ~1MB DMA,
# small last chunk so the final output DMA tail is short.
CHUNKS = [8, 16, 240, 232, 16]
NCHUNK = len(CHUNKS)
CSTART = [sum(CHUNKS[:i]) for i in range(NCHUNK)]

PAD = 8                     # trailing dummy cols on tight-pair producers

ALPHA_MEM = float(np.exp(-1.0 / 20.0))
ALPHA_ADP = float(np.exp(-1.0 / 200.0))
# x~ bias: fold the compare constant 2 into the membrane state.
XBIAS = float(np.float32(2.0 * np.float32(ALPHA_MEM) - 2.0))
# K_t = alpha_mem^-t at fp32 (matches the numpy validation exactly).
KPOW = (np.float64(np.float32(ALPHA_MEM)) ** (-np.arange(T))).astype(np.float32)
# n_{-1} = g_init * K_{-1} = (-2) * alpha_mem
NINIT = float(np.float32(-2.0) * np.float32(ALPHA_MEM))

LAST_RESULT = None  # BassKernelResults of the most recent run (for test.py)

F32 = mybir.dt.float32
OP = mybir.AluOpType


def _build():
    nc = bacc.Bacc("TRN2", target_bir_lowering=False, debug=False)
    x_ext = nc.declare_dram_parameter("x", [EH, T * EL], F32, isOutput=False)
    s_ext = nc.declare_dram_parameter("out", [EH, T * EL], F32, isOutput=True)

    with tile.TileContext(nc) as tc, ExitStack() as ctx:
        data = ctx.enter_context(tc.tile_pool(name="data", bufs=1))
        xin = [data.tile([EH, CHUNKS[k] * EL + PAD], F32, name=f"xin{k}", tag=f"x{k}")
               for k in range(NCHUNK)]
        sout = [data.tile([EH, CHUNKS[k] * EL + PAD], F32, name=f"sout{k}", tag=f"s{k}")
                for k in range(NCHUNK)]

        st = ctx.enter_context(tc.tile_pool(name="state", bufs=1))
        n = st.tile([EH, EL + PAD], F32, tag="n")
        a = st.tile([EH, EL + PAD], F32, tag="a")
        fb = st.tile([EH, EL + PAD], F32, tag="fb")
        szero = st.tile([EH, EL + PAD], F32, tag="szero")
        bias = st.tile([EH, 1], F32, tag="bias")
        nc.vector.memset(bias[:], XBIAS)

        for k in range(NCHUNK):
            nc.sync.dma_start(
                xin[k][:, 0:CHUNKS[k] * EL],
                x_ext[:, CSTART[k] * EL:(CSTART[k] + CHUNKS[k]) * EL])
            nc.vector.memset(xin[k][:, CHUNKS[k] * EL:], 0.0)
            # x~ = x + c0 on the Scalar engine (own SBUF port, otherwise
            # idle); one instruction per chunk, pipelined ahead of the DVE.
            nc.scalar.activation(xin[k][:, 0:CHUNKS[k] * EL],
                                 xin[k][:, 0:CHUNKS[k] * EL],
                                 mybir.ActivationFunctionType.Identity,
                                 bias=bias[:], scale=1.0)

        nc.vector.memset(n[:], NINIT)
        nc.vector.memset(a[:], 0.0)
        nc.vector.memset(szero[:], 0.0)

        # Per-step group [F(t), A(t-1), R(t), CMP(t)].  The only tight
        # (adjacent-instruction) same-engine dependency is R->CMP, whose
        # producer streams PAD extra dummy columns so its real writebacks
        # retire before CMP's reads reach them -- replacing semaphore
        # waits.  All other deps are >=2 instructions back, where the DVE
        # pipeline overlap can no longer race (empirically validated), so
        # their semaphore waits are stripped below.
        def slot(t):
            k = next(i for i in range(NCHUNK)
                     if CSTART[i] <= t < CSTART[i] + CHUNKS[i])
            return k, t - CSTART[k]

        for t in range(T):
            k, j = slot(t)
            xt_pad = xin[k][:, j * EL:(j + 1) * EL + PAD]
            st_ = sout[k][:, j * EL:(j + 1) * EL]
            if t == 0:
                sprev_pad = szero[:]
            else:
                kp, jp = slot(t - 1)
                sprev_pad = sout[kp][:, jp * EL:jp * EL + EL + PAD]
            kt = float(KPOW[t])
            kprev = float(ALPHA_MEM) if t == 0 else float(KPOW[t - 1])
            ck = float(np.float32(np.float32(0.1) * KPOW[t]))

            # F = K_t * x~_t + n_{t-1}   (off the spike cycle; FD=EL only --
            # R reads fb's stale pad cols, which is harmless garbage)
            nc.vector.scalar_tensor_tensor(fb[:, 0:EL], xin[k][:, j * EL:(j + 1) * EL],
                                           kt, n[:, 0:EL],
                                           op0=OP.mult, op1=OP.add)
            # a-update for the previous step (s_{t-1} is 2 insts back)
            if t > 0:
                nc.vector.scalar_tensor_tensor(a[:, 0:EL], a[:, 0:EL],
                                               ALPHA_ADP,
                                               sout[kp][:, jp * EL:(jp + 1) * EL],
                                               op0=OP.mult, op1=OP.add)
            # n = F - K_{t-1} * s_{t-1}  (reset; streams PAD cols so CMP
            # can follow immediately without a wait)
            nc.vector.scalar_tensor_tensor(n[:], sprev_pad, -kprev, fb[:],
                                           op0=OP.mult, op1=OP.add)
            # s = (0.1*K_t*a <= n) -> output buffer
            nc.vector.scalar_tensor_tensor(st_, a[:, 0:EL], ck, n[:, 0:EL],
                                           op0=OP.mult, op1=OP.is_le)

            if j == CHUNKS[k] - 1:
                if k == NCHUNK - 1:
                    # Tail chunk: split across two queues (partition halves)
                    # so the final, latency-exposed transfer is halved.
                    nc.sync.dma_start(
                        s_ext[0:EH // 2,
                              CSTART[k] * EL:(CSTART[k] + CHUNKS[k]) * EL],
                        sout[k][0:EH // 2, 0:CHUNKS[k] * EL])
                    nc.sync.dma_start(
                        s_ext[EH // 2:EH,
                              CSTART[k] * EL:(CSTART[k] + CHUNKS[k]) * EL],
                        sout[k][EH // 2:EH, 0:CHUNKS[k] * EL])
                else:
                    nc.sync.dma_start(
                        s_ext[:, CSTART[k] * EL:(CSTART[k] + CHUNKS[k]) * EL],
                        sout[k][:, 0:CHUNKS[k] * EL])

    _strip_dve_sem_overhead(nc)
    nc.finalize()
    return nc


def _strip_dve_sem_overhead(nc):
    # The DVE overlaps at most the next instruction with the current one,
    # so a RAW hazard only exists between ADJACENT DVE instructions, and
    # the PAD trailing columns on the producer of the single tight pair
    # (R->CMP) delay the consumer's reads past the producer's writebacks.
    # That makes every Tile-emitted DVE-on-DVE semaphore wait (~180ns
    # event-propagation latency each) redundant -- strip them all.
    # Cross-engine waits (DMA/ACT<->DVE) and all semaphore updates are kept.
    for f in nc.m.functions:
        for bb in f.blocks:
            for inst in bb.instructions:
                if inst.engine != mybir.EngineType.DVE:
                    continue
                si = inst.sync_info
                if si is not None and si.on_wait:
                    kept = [w for w in si.on_wait
                            if not str(w.ant_name).startswith("DVE")]
                    if len(kept) != len(si.on_wait):
                        si.on_wait = kept

    # Of the ~2050 DVE semaphore updates only a handful of cumulative
    # threshold values are ever awaited (output DMAs, kernel-tail drain,
    # barrier event-semaphores).  Drop the updates nobody waits for and
    # remap the awaited thresholds to the compressed count, removing the
    # per-instruction semaphore-update overhead from the hot loop.
    insts = [i for f in nc.m.functions for bb in f.blocks for i in bb.instructions]

    def dve_sem_names(entries):
        return {str(e.ant_name) for e in entries if str(e.ant_name).startswith("DVE")}

    sems = set()
    for i in insts:
        if i.sync_info:
            sems |= dve_sem_names(i.sync_info.on_update or [])
    for sem in sems:
        awaited = set()
        for i in insts:
            si = i.sync_info
            if si is None:
                continue
            for wt in (si.on_wait or []):
                if str(wt.ant_name) == sem:
                    awaited.add(wt.wait_value)
        ordinal = 0
        remap = {}
        kept_count = 0
        for i in insts:
            si = i.sync_info
            if si is None:
                continue
            ups = [u for u in (si.on_update or []) if str(u.ant_name) == sem]
            if not ups:
                continue
            ordinal += 1
            if ordinal in awaited:
                kept_count += 1
                remap[ordinal] = kept_count
            else:
                si.on_update = [u for u in si.on_update
                                if str(u.ant_name) != sem]
        for i in insts:
            si = i.sync_info
            if si is None:
                continue
            for wt in (si.on_wait or []):
                if str(wt.ant_name) == sem:
                    wt.wait_value = remap[wt.wait_value]


def kernel(x: np.ndarray) -> np.ndarray:
    global LAST_RESULT
    x = np.ascontiguousarray(x, dtype=np.float32)
    assert x.shape == (B, T, D)

    # shard: core c owns d in [c*DLOC, (c+1)*DLOC); element (b, dh, dl):
    # eh = b*4 + dh, el = dl  with d = c*128 + dh*32 + dl
    xs = (x.reshape(B, T, NCORES, EH // B, EL)
           .transpose(2, 0, 3, 1, 4)
           .reshape(NCORES, EH, T * EL))

    nc = _build()
    in_maps = [{"x": np.ascontiguousarray(xs[c])} for c in range(NCORES)]
    LAST_RESULT = run_bass_kernel_spmd(
        nc, in_maps, list(range(NCORES)),
        trace=bool(os.environ.get("ADLIF_TRACE")),
    )
    outs = np.stack([LAST_RESULT.results[c]["out"] for c in range(NCORES)])

    s = (outs.reshape(NCORES, B, EH // B, T, EL)
             .transpose(1, 3, 0, 2, 4)
             .reshape(B, T, D))
    return np.ascontiguousarray(s, dtype=np.float32)


# revision 15
# speedup vs baseline: 1.0959x; 1.0437x over previous
"""AdLIF neuron Bass kernel for 8 Trainium2 NeuronCores.

2-hop-dependency formulation in decay-rescaled space.  With
m_t = u_t - 2 (pre-reset shifted membrane) and K_t = alpha_mem^-t,
track n_t = m_t * K_t.  The recurrence becomes a running sum:
    F_t = n_{t-1} + K_t * x~_t          x~ = x + 2*(alpha_mem - 1)
    n_t = F_t - K_{t-1} * s_{t-1}       (reset)
    s_t = (0.1 * K_t * a_{t-1} <= n_t)  (<=> u_t >= 0.1*a + 2)
    a_t = alpha_adp * a_{t-1} + s_t
All K powers enter as per-instruction scalar immediates, so every
dependency cycle through the spike decision is 2 instructions long
(F depends only on n, not on s).  Numerically validated vs the jax
reference (0 mismatches in fp32 emulation on the actual input).
The x~ pre-shift runs on the Scalar (Activation) engine per chunk,
hidden behind the DVE recurrence.

Sharding: D (1024) split across 8 cores -> 128 d's per core.
Per core the 32*128 = 4096 (b,d) elements are laid out as
[eh=128 partitions, el=32 free] and time runs in the free dim of a
[128, T*32] SBUF buffer, so each timestep is one [128, 32] slice.
Host pre-reshapes x to [core, eh, t, el] so the per-core DMA is one
fully contiguous 64KB-per-partition stream.
"""

import os
import numpy as np
from contextlib import ExitStack

import concourse.bass as bass
import concourse.tile as tile
from concourse import bacc, mybir
from concourse.bass_utils import run_bass_kernel_spmd

B, T, D = 32, 512, 1024
NCORES = 8
DLOC = D // NCORES          # 128 d's per core
EH, EL = 128, 32            # 4096 elements per core = EH partitions x EL free
# Uneven t-chunks: small first chunk so compute starts after a ~1MB DMA,
# small last chunk so the final output DMA tail is short.
CHUNKS = [8, 16, 32, 64, 96, 136, 144, 16]
NCHUNK = len(CHUNKS)
CSTART = [sum(CHUNKS[:i]) for i in range(NCHUNK)]

PAD = 8                     # trailing dummy cols on tight-pair producers

ALPHA_MEM = float(np.exp(-1.0 / 20.0))
ALPHA_ADP = float(np.exp(-1.0 / 200.0))
# x~ bias: fold the compare constant 2 into the membrane state.
XBIAS = float(np.float32(2.0 * np.float32(ALPHA_MEM) - 2.0))
# K_t = alpha_mem^-t at fp32 (matches the numpy validation exactly).
KPOW = (np.float64(np.float32(ALPHA_MEM)) ** (-np.arange(T))).astype(np.float32)
# n_{-1} = g_init * K_{-1} = (-2) * alpha_mem
NINIT = float(np.float32(-2.0) * np.float32(ALPHA_MEM))

LAST_RESULT = None  # BassKernelResults of the most recent run (for test.py)

F32 = mybir.dt.float32
OP = mybir.AluOpType


def _build():
    nc = bacc.Bacc("TRN2", target_bir_lowering=False, debug=False)
    x_ext = nc.declare_dram_parameter("x", [EH, T * EL], F32, isOutput=False)
    s_ext = nc.declare_dram_parameter("out", [EH, T * EL], F32, isOutput=True)

    with tile.TileContext(nc) as tc, ExitStack() as ctx:
        data = ctx.enter_context(tc.tile_pool(name="data", bufs=1))
        xin = [data.tile([EH, CHUNKS[k] * EL + PAD], F32, name=f"xin{k}", tag=f"x{k}")
               for k in range(NCHUNK)]
        sout = [data.tile([EH, CHUNKS[k] * EL + PAD], F32, name=f"sout{k}", tag=f"s{k}")
                for k in range(NCHUNK)]

        st = ctx.enter_context(tc.tile_pool(name="state", bufs=1))
        n = st.tile([EH, EL + PAD], F32, tag="n")
        a = st.tile([EH, EL + PAD], F32, tag="a")
        fb = st.tile([EH, EL + PAD], F32, tag="fb")
        szero = st.tile([EH, EL + PAD], F32, tag="szero")
        bias = st.tile([EH, 1], F32, tag="bias")
        warm = st.tile([EH, 1], F32, tag="warm")
        nc.vector.memset(bias[:], XBIAS)
        # Dependency-free dummy activation: pulls the Identity table load
        # to kernel start so it doesn't serialize after the first DMA.
        nc.scalar.activation(warm[:], warm[:],
                             mybir.ActivationFunctionType.Identity,
                             bias=bias[:], scale=1.0)

        for k in range(NCHUNK):
            nc.sync.dma_start(
                xin[k][:, 0:CHUNKS[k] * EL],
                x_ext[:, CSTART[k] * EL:(CSTART[k] + CHUNKS[k]) * EL])
            nc.vector.memset(xin[k][:, CHUNKS[k] * EL:], 0.0)
            # x~ = x + c0 on the Scalar engine (own SBUF port, otherwise
            # idle); one instruction per chunk, pipelined ahead of the DVE.
            nc.scalar.activation(xin[k][:, 0:CHUNKS[k] * EL],
                                 xin[k][:, 0:CHUNKS[k] * EL],
                                 mybir.ActivationFunctionType.Identity,
                                 bias=bias[:], scale=1.0)

        nc.vector.memset(n[:], NINIT)
        nc.vector.memset(a[:], 0.0)
        nc.vector.memset(szero[:], 0.0)

        # Per-step group [F(t), A(t-1), R(t), CMP(t)].  The only tight
        # (adjacent-instruction) same-engine dependency is R->CMP, whose
        # producer streams PAD extra dummy columns so its real writebacks
        # retire before CMP's reads reach them -- replacing semaphore
        # waits.  All other deps are >=2 instructions back, where the DVE
        # pipeline overlap can no longer race (empirically validated), so
        # their semaphore waits are stripped below.
        def slot(t):
            k = next(i for i in range(NCHUNK)
                     if CSTART[i] <= t < CSTART[i] + CHUNKS[i])
            return k, t - CSTART[k]

        for t in range(T):
            k, j = slot(t)
            xt_pad = xin[k][:, j * EL:(j + 1) * EL + PAD]
            st_ = sout[k][:, j * EL:(j + 1) * EL]
            if t == 0:
                sprev_pad = szero[:]
            else:
                kp, jp = slot(t - 1)
                sprev_pad = sout[kp][:, jp * EL:jp * EL + EL + PAD]
            kt = float(KPOW[t])
            kprev = float(ALPHA_MEM) if t == 0 else float(KPOW[t - 1])
            ck = float(np.float32(np.float32(0.1) * KPOW[t]))

            # F = K_t * x~_t + n_{t-1}   (off the spike cycle; FD=EL only --
            # R reads fb's stale pad cols, which is harmless garbage)
            nc.vector.scalar_tensor_tensor(fb[:, 0:EL], xin[k][:, j * EL:(j + 1) * EL],
                                           kt, n[:, 0:EL],
                                           op0=OP.mult, op1=OP.add)
            # a-update for the previous step (s_{t-1} is 2 insts back)
            if t > 0:
                nc.vector.scalar_tensor_tensor(a[:, 0:EL], a[:, 0:EL],
                                               ALPHA_ADP,
                                               sout[kp][:, jp * EL:(jp + 1) * EL],
                                               op0=OP.mult, op1=OP.add)
            # n = F - K_{t-1} * s_{t-1}  (reset; streams PAD cols so CMP
            # can follow immediately without a wait)
            nc.vector.scalar_tensor_tensor(n[:], sprev_pad, -kprev, fb[:],
                                           op0=OP.mult, op1=OP.add)
            # s = (0.1*K_t*a <= n) -> output buffer
            nc.vector.scalar_tensor_tensor(st_, a[:, 0:EL], ck, n[:, 0:EL],
                                           op0=OP.mult, op1=OP.is_le)

            if j == CHUNKS[k] - 1:
                if k == NCHUNK - 1:
                    # Tail chunk: split across two queues (partition halves)
                    # so the final, latency-exposed transfer is halved.
                    nc.sync.dma_start(
                        s_ext[0:EH // 2,
                              CSTART[k] * EL:(CSTART[k] + CHUNKS[k]) * EL],
                        sout[k][0:EH // 2, 0:CHUNKS[k] * EL])
                    nc.sync.dma_start(
                        s_ext[EH // 2:EH,
                              CSTART[k] * EL:(CSTART[k] + CHUNKS[k]) * EL],
                        sout[k][EH // 2:EH, 0:CHUNKS[k] * EL])
                else:
                    nc.sync.dma_start(
                        s_ext[:, CSTART[k] * EL:(CSTART[k] + CHUNKS[k]) * EL],
                        sout[k][:, 0:CHUNKS[k] * EL])

    _strip_dve_sem_overhead(nc)
    nc.finalize()
    return nc


def _strip_dve_sem_overhead(nc):
    # The DVE overlaps at most the next instruction with the current one,
    # so a RAW hazard only exists between ADJACENT DVE instructions, and
    # the PAD trailing columns on the producer of the single tight pair
    # (R->CMP) delay the consumer's reads past the producer's writebacks.
    # That makes every Tile-emitted DVE-on-DVE semaphore wait (~180ns
    # event-propagation latency each) redundant -- strip them all.
    # Cross-engine waits (DMA/ACT<->DVE) and all semaphore updates are kept.
    for f in nc.m.functions:
        for bb in f.blocks:
            for inst in bb.instructions:
                if inst.engine != mybir.EngineType.DVE:
                    continue
                si = inst.sync_info
                if si is not None and si.on_wait:
                    kept = [w for w in si.on_wait
                            if not str(w.ant_name).startswith("DVE")]
                    if len(kept) != len(si.on_wait):
                        si.on_wait = kept

    # Of the ~2050 DVE semaphore updates only a handful of cumulative
    # threshold values are ever awaited (output DMAs, kernel-tail drain,
    # barrier event-semaphores).  Drop the updates nobody waits for and
    # remap the awaited thresholds to the compressed count, removing the
    # per-instruction semaphore-update overhead from the hot loop.
    insts = [i for f in nc.m.functions for bb in f.blocks for i in bb.instructions]

    def dve_sem_names(entries):
        return {str(e.ant_name) for e in entries if str(e.ant_name).startswith("DVE")}

    sems = set()
    for i in insts:
        if i.sync_info:
            sems |= dve_sem_names(i.sync_info.on_update or [])
    for sem in sems:
        awaited = set()
        for i in insts:
            si = i.sync_info
            if si is None:
                continue
            for wt in (si.on_wait or []):
                if str(wt.ant_name) == sem:
                    awaited.add(wt.wait_value)
        ordinal = 0
        remap = {}
        kept_count = 0
        for i in insts:
            si = i.sync_info
            if si is None:
                continue
            ups = [u for u in (si.on_update or []) if str(u.ant_name) == sem]
            if not ups:
                continue
            ordinal += 1
            if ordinal in awaited:
                kept_count += 1
                remap[ordinal] = kept_count
            else:
                si.on_update = [u for u in si.on_update
                                if str(u.ant_name) != sem]
        for i in insts:
            si = i.sync_info
            if si is None:
                continue
            for wt in (si.on_wait or []):
                if str(wt.ant_name) == sem:
                    wt.wait_value = remap[wt.wait_value]


def kernel(x: np.ndarray) -> np.ndarray:
    global LAST_RESULT
    x = np.ascontiguousarray(x, dtype=np.float32)
    assert x.shape == (B, T, D)

    # shard: core c owns d in [c*DLOC, (c+1)*DLOC); element (b, dh, dl):
    # eh = b*4 + dh, el = dl  with d = c*128 + dh*32 + dl
    xs = (x.reshape(B, T, NCORES, EH // B, EL)
           .transpose(2, 0, 3, 1, 4)
           .reshape(NCORES, EH, T * EL))

    nc = _build()
    in_maps = [{"x": np.ascontiguousarray(xs[c])} for c in range(NCORES)]
    LAST_RESULT = run_bass_kernel_spmd(
        nc, in_maps, list(range(NCORES)),
        trace=bool(os.environ.get("ADLIF_TRACE")),
    )
    outs = np.stack([LAST_RESULT.results[c]["out"] for c in range(NCORES)])

    s = (outs.reshape(NCORES, B, EH // B, T, EL)
             .transpose(1, 3, 0, 2, 4)
             .reshape(B, T, D))
    return np.ascontiguousarray(s, dtype=np.float32)
